# revision 49
# baseline (speedup 1.0000x reference)
"""Trainium2 Bass kernel for ConvNdFunc: 16x16/stride-8 patch MLP (256->1024->1).

Data-parallel over batch: 32 images -> 8 cores x 4 images, no collectives.

Host pre-computes a dense im2col phase layout, one buffer per K-chunk c,
flat over all 63 window-rows: xd[b, p, c, wr*63+wc] = x[b, 8*wr + p//8,
8*wc + 8c + p%8] (p = kh*8+kwp). A tile (7 window-rows x 63 cols = 441
windows) is just a column range of this buffer; both L1 chunk matmuls read
flat stride-1 rhs APs. No on-chip im2col, no padding waste.

Per tile (PE period ~3.47us, zero steady-state PE gaps):
  - L1 (TensorE): ht[hid128, 441] += W1_chunk.T @ patches, 8 hidden blocks x
    2 K-chunks = 16 bf16 matmuls (f32 PSUM accum, 6-bank pipeline).
  - ReLU PSUM -> bf16 SBUF: 5 blocks on ScalarE activation, 3 on VectorE max
    (L2 is software-pipelined a tile behind).
  - L2 (TensorE): 8 accumulating M=1 matmuls on 4 concurrent column strips
    (tile_position (0,0)/(0,32)/(0,64)/(0,96)), 2 per strip -> ~2 slots per
    tile; emitted two tiles at a time so the full-width<->strip weight-load
    transition (~92ns, row_grp conflict with in-flight L1 matmuls) is paid
    once per two tiles.
  - Merge: DVE ops allow one PSUM operand and only 32-multiple partition
    shifts, so: u[0:33] = ops[64:97] + b2/2 (stage), v2[0:33,col] =
    ops[0:33] + u (lanes 0/32 hold the two partials; col indexes the tile
    pair), then fold lanes with two accumulating gpsimd SWDGE DMAs over the
    contiguous 2-tile y span (zero-initialized output; consecutive tiles'
    y blocks are DRAM-contiguous, incl. the image seam). The last 3 tiles
    use a 2-strip L2 + stage/fused-STT + direct sync-ring DMA instead, so
    no SWDGE accum lands near the kernel end (its queue drain would gate
    the fixed ~8us epilogue); the very last tile runs as two independent
    column halves so the left half's merge + store overlap the right
    half's matmuls (~0.7us shorter serial tail).
  - Head: DMA queues can't start before the ~7.2us engine preamble and run
    at a cold clock after; tile-0's critical set (x0 both chunks + w1 hb0 +
    hb7) rides the sync queue, hb1-3+w2 gpsimd, hb4-6 scalar. 14x320-col +
    8x128-col zero matmuls bridge the PE from preamble to tile-0 data so
    the HAM clock-gate warms before real L1 (no 1.2GHz chilled region and
    no mid-kernel re-throttle).

History: inherited 160.9us; this version measures ~147.3-147.5us (bf16 L1
is at the PE roofline ~107us; ~15us is fixed preamble + epilogue ceremony).
rel err ~3.4e-3 (bf16 data path, f32 accumulate).
"""

import os
import sys
from contextlib import ExitStack

_RT = "/opt/trn_rl_repo"
if _RT not in sys.path:
    sys.path.insert(0, _RT)

import ml_dtypes
import numpy as np

def _ensure_ntff_hook():
    """Register the axon NTFF profiling hook if the image's antenv lacks it.

    Only matters when tracing (KERNEL_TRACE=1); no-op side effects otherwise.
    """
    import types

    try:
        import antenv.axon_hooks  # noqa: F401

        return
    except ImportError:
        pass
    try:
        import antenv
    except ImportError:
        return
    mod = types.ModuleType("antenv.axon_hooks")
    _state = {"hook": None}
    mod.set_axon_ntff_profile_hook = lambda h: _state.__setitem__("hook", h)
    mod.get_axon_ntff_profile_hook = lambda: _state["hook"]
    sys.modules["antenv.axon_hooks"] = mod
    antenv.axon_hooks = mod
    try:
        from trn_agent_boot.trn_boot import _ntff_profile_via_ctypes

        mod.set_axon_ntff_profile_hook(
            _ntff_profile_via_ctypes("/opt/axon/libaxon_pjrt.so")
        )
    except Exception:
        pass


_ensure_ntff_hook()

import concourse.bass as bass
import concourse.tile as tile
from concourse import bacc, mybir
from concourse.bass_utils import run_bass_kernel_spmd

B, H, W = 32, 512, 512
KK, S, HID = 16, 8, 1024
OH = OW = (H - KK) // S + 1  # 63
NCORES = 8
BPC = B // NCORES  # 4 images per core
NWP = 7 * OW  # matmul free dim per tile (7 window-rows x 63 cols)
NHB = HID // 128  # 8 hidden blocks
# uniform 7-row groups measured faster than variable-size groups (a small
# 3-row first + 4-row last tile saved head/tail latency but the 4 extra
# tiles' transitions/overheads cost ~5us net)
G_SIZES = [7] * 9
assert sum(G_SIZES) == OH
GROUPS = []  # (col0, width) in the per-image flat window space
_c = 0
for _n in G_SIZES:
    GROUPS.append((_c, _n * OW))
    _c += _n * OW
NG = len(GROUPS)  # 9 tiles per image
NWIN = OH * OW  # 3969 windows per image

BF16 = ml_dtypes.bfloat16
F32 = mybir.dt.float32
BF16_T = mybir.dt.bfloat16

LAST_RESULTS = None  # BassKernelResults of the most recent run (for test harness)

HB_ORDER = list(range(NHB))
SCAL_HB = {0, 1, 2, 3, 4}  # ReLU on ScalarE; rest on VectorE (L2 is pipelined
# one tile behind, so ReLU completion order no longer gates L2 pairs)
N_WARM = 14  # ~267ns each (320-col): bridge PE from preamble to tile-0 data
# so the HAM clock-gate warms during the bridge, not 3.4us into real L1;
# a fine-grained 128-col bridge tail follows (see below)
N_WARM_COLS = 320
BANK = 512  # PSUM bank stride in fp32 elements
N_TAIL = 3  # last tiles use the direct (sync-DMA) merge to avoid SWDGE latency


def _build_nc(b2_val: float, b1_nonzero: bool):
    nc = bacc.Bacc(None, target_bir_lowering=False)

    # host dense phase layout per K-chunk c, flat over all 63 window-rows:
    # x[b, p, c, wr*63+wc] = img[8*wr + p//8, 8*wc + 8c + p%8] (p = kh*8+kwp);
    # a group is just a column range of this buffer
    x_d = nc.dram_tensor("x", [BPC, 128, 2, NWIN], BF16_T, kind="ExternalInput")
    w1_d = nc.dram_tensor("w1", [128, 2, HID], BF16_T, kind="ExternalInput")
    w2_d = nc.dram_tensor("w2", [128, NHB], BF16_T, kind="ExternalInput")
    b1_d = nc.dram_tensor("b1", [1, HID], BF16_T, kind="ExternalInput")
    y_d = nc.dram_tensor("y", [BPC, OH, OW], F32, kind="ExternalOutput")

    relu = mybir.ActivationFunctionType.Relu

    with tile.TileContext(nc) as tc, ExitStack() as ctx:
        consts = ctx.enter_context(tc.tile_pool(name="consts", bufs=1))
        xin_pool = ctx.enter_context(tc.tile_pool(name="xin", bufs=4))
        hs_pool = ctx.enter_context(tc.tile_pool(name="hs", bufs=4))
        osb_pool = ctx.enter_context(tc.tile_pool(name="osb", bufs=8))
        ht_pool = ctx.enter_context(tc.tile_pool(name="ht", bufs=6, space="PSUM"))
        ops_pool = ctx.enter_context(tc.tile_pool(name="ops", bufs=2, space="PSUM"))

        w1_sb = consts.tile([128, 2, HID], BF16_T)
        w2_sb = consts.tile([128, NHB], BF16_T)

        # PE runs at a cold clock for the first ~5us of activity; a few zero
        # matmuls start the HAM ramp while the first DMAs are in flight.
        warm_in = consts.tile([128, 512], BF16_T)
        nc.gpsimd.memset(warm_in, 0.0)
        warm_ps = ht_pool.tile([128, NWP], F32, tag="ht")
        for _ in range(N_WARM):
            nc.tensor.matmul(
                warm_ps[:, 0:N_WARM_COLS],
                warm_in[:, 0:128],
                warm_in[:, 0:N_WARM_COLS],
                start=True,
                stop=True,
            )
        # fine-grained bridge tail (~107ns each cold): when tile-0 data lands
        # early, the leftover bridge costs little; when late, PE stays busy so
        # the HAM clock-gate doesn't re-arm
        for _ in range(8):
            nc.tensor.matmul(
                warm_ps[:, 0:128],
                warm_in[:, 0:128],
                warm_in[:, 0:128],
                start=True,
                stop=True,
            )
        if b1_nonzero:
            b1_sb = consts.tile([1, HID], BF16_T)
            nc.scalar.dma_start(out=b1_sb, in_=b1_d[:, :])
            ones_sb = consts.tile([1, NWP], BF16_T)
            nc.vector.memset(ones_sb, 1.0)

        first = True

        y_flat = y_d.reshape([BPC * NWIN])

        def emit_l2_mms(prev, v2, vc0):
            # L2 strip matmuls for one tile + the 2-op DVE merge into columns
            # [vc0, vc0+w) of the shared pair buffer v2 [33, 2*NWP]. Four
            # concurrent column strips (PE col groups 0/32/64/96, psum
            # partitions likewise), two accumulating matmuls per strip.
            b, c0, w, hs = prev
            ops = ops_pool.tile([97, NWP], F32)
            for k in range(2):
                for s_i, part in enumerate((0, 32, 64, 96)):
                    hb = 4 * k + s_i
                    nc.tensor.matmul(
                        ops[part : part + 1, 0:w],
                        w2_sb[:, hb : hb + 1],
                        hs[:, hb, 0:w],
                        start=(k == 0),
                        stop=(k == 1),
                        tile_position=(0, part),
                    )
            # merge on VectorE (one PSUM operand per DVE op; partition shifts
            # must be multiples of 32): stage {64,96} down by 64 with +b2/2
            # each lane (b2 lands twice across the two lanes), then add slab
            # {0..32}. v2 lanes 0/32 hold the two partial sums.
            u = osb_pool.tile([33, NWP], F32, tag="u")
            nc.vector.tensor_scalar_add(
                u[:, 0:w], ops[64:97, 0:w], float(b2_val) * 0.5
            )
            nc.vector.tensor_tensor(
                v2[:, vc0 : vc0 + w], ops[0:33, 0:w], u[:, 0:w],
                mybir.AluOpType.add,
            )

        def emit_l2_pair(prevA, prevB):
            # two consecutive tiles' y blocks are contiguous in DRAM (groups
            # per image are even, so pairs never cross an image seam), so
            # fold both tiles' lanes 0/32 with two accumulating SWDGE DMAs
            # onto the zero-initialized 2-tile y span (same gpsimd queue ->
            # ordered read-modify-write); keeps y off the strict-FIFO sync
            # ring so x prefetches never queue behind a fold
            bA, c0A, wA, _ = prevA
            wB = prevB[2]
            v2 = osb_pool.tile([33, 2 * NWP], F32, tag="v2")
            emit_l2_mms(prevA, v2, 0)
            emit_l2_mms(prevB, v2, wA)
            pstep = v2.ap[0][0]
            off = bA * NWIN + c0A
            wT = wA + wB
            for row in (0, 32):
                src = bass.AP(
                    tensor=v2.tensor,
                    offset=v2.offset + row * pstep,
                    ap=[[pstep, 1], [1, wT]],
                )
                nc.gpsimd.dma_start(
                    out=y_flat[off : off + wT],
                    in_=src,
                    accum_op=mybir.AluOpType.add,
                )

        def emit_l2(prev, halves=1):
            # kernel-tail path: 2-strip L2 (shorter post-L2 merge chain than
            # 4 strips: stage + fused STT), direct sync-ring store -- no
            # SWDGE accum near the end (its queue drain gates the epilogue).
            # With halves=2 (the very last tile) the L2/merge/store run as
            # two independent column halves so the left half's merge + DMA
            # overlap the right half's matmuls (~0.7us shorter tail).
            b, c0, w, hs = prev
            ops = ops_pool.tile([33, NWP], F32)
            bounds = [(0, w)] if halves == 1 else [(0, w // 2), (w // 2, w)]
            for h0, h1 in bounds:
                for k in range(NHB // 2):
                    nc.tensor.matmul(
                        ops[0:1, h0:h1],
                        w2_sb[:, 2 * k : 2 * k + 1],
                        hs[:, 2 * k, h0:h1],
                        start=(k == 0),
                        stop=(k == NHB // 2 - 1),
                        tile_position=(0, 0),
                    )
                    nc.tensor.matmul(
                        ops[32:33, h0:h1],
                        w2_sb[:, 2 * k + 1 : 2 * k + 2],
                        hs[:, 2 * k + 1, h0:h1],
                        start=(k == 0),
                        stop=(k == NHB // 2 - 1),
                        tile_position=(0, 32),
                    )
                hw = h1 - h0
                o32 = osb_pool.tile([1, NWP], F32, tag="o32")
                nc.vector.tensor_scalar_add(
                    o32[:, 0:hw], ops[32:33, h0:h1], 0.0
                )
                osb = osb_pool.tile([1, NWP], F32)
                nc.vector.scalar_tensor_tensor(
                    osb[:, 0:hw],
                    ops[0:1, h0:h1],
                    float(b2_val),
                    o32[:, 0:hw],
                    mybir.AluOpType.add,
                    mybir.AluOpType.add,
                )
                nc.sync.dma_start(
                    out=y_flat[b * NWIN + c0 + h0 : b * NWIN + c0 + h1],
                    in_=osb[0:1, 0:hw],
                )

        pend = []
        for b in range(BPC):
            for gi, (c0, w) in enumerate(GROUPS):
                xin = xin_pool.tile([128, 2, NWP], BF16_T)
                if first:
                    # cold-window DMA plan: tile-0's whole critical set (the
                    # small 3-row first group, both chunks, + w1 hb0) rides
                    # the sync queue, which starts earliest and ramps fastest.
                    # gpsimd carries hb1-3 (+w2), scalar hb4-6, sync hb7.
                    nc.sync.dma_start(
                        out=xin[:, 0, 0:w], in_=x_d[b, :, 0, c0 : c0 + w]
                    )
                    nc.sync.dma_start(
                        out=xin[:, 1, 0:w], in_=x_d[b, :, 1, c0 : c0 + w]
                    )
                    nc.sync.dma_start(
                        out=w1_sb[:, :, 0:128], in_=w1_d[:, :, 0:128]
                    )
                    for lo in range(128, 512, 128):
                        nc.gpsimd.dma_start(
                            out=w1_sb[:, :, lo : lo + 128],
                            in_=w1_d[:, :, lo : lo + 128],
                        )
                    for lo in range(512, 896, 128):
                        nc.scalar.dma_start(
                            out=w1_sb[:, :, lo : lo + 128],
                            in_=w1_d[:, :, lo : lo + 128],
                        )
                    nc.sync.dma_start(
                        out=w1_sb[:, :, 896:HID], in_=w1_d[:, :, 896:HID]
                    )
                    nc.gpsimd.dma_start(out=w2_sb, in_=w2_d[:, :])
                else:
                    nc.sync.dma_start(
                        out=xin[:, :, 0:w], in_=x_d[b, :, :, c0 : c0 + w]
                    )
                first = False

                hs = hs_pool.tile([128, NHB, NWP], BF16_T)
                for hb in HB_ORDER:
                    ht = ht_pool.tile([128, NWP], F32)
                    if b1_nonzero:
                        nc.tensor.matmul(
                            ht[:, 0:w],
                            b1_sb[:, hb * 128 : (hb + 1) * 128],
                            ones_sb[:, 0:w],
                            start=True,
                            stop=False,
                        )
                    for c in range(2):
                        nc.tensor.matmul(
                            ht[:, 0:w],
                            w1_sb[:, c, hb * 128 : (hb + 1) * 128],
                            xin[:, c, 0:w],
                            start=(c == 0 and not b1_nonzero),
                            stop=(c == 1),
                        )
                    if hb in SCAL_HB:
                        nc.scalar.activation(
                            out=hs[:, hb, 0:w], in_=ht[:, 0:w], func=relu
                        )
                    else:
                        nc.vector.tensor_scalar_max(
                            hs[:, hb, 0:w], ht[:, 0:w], 0.0
                        )

                # batch the pipelined L2s two tiles at a time: one strip
                # transition (~280ns of PE drain-wait) per two tiles
                pend.append((b, c0, w, hs))
                t = b * NG + gi
                if t >= BPC * NG - 3:
                    # near the kernel tail: flush immediately via the direct
                    # 2-strip merge path so the final merges + output DMAs
                    # overlap remaining compute and no SWDGE accum lands near
                    # the end (its queue drain would gate the epilogue)
                    while pend:
                        p_ = pend.pop(0)
                        last = p_[0] == BPC - 1 and p_[1] == GROUPS[-1][0]
                        emit_l2(p_, halves=2 if last else 1)
                elif len(pend) >= 2 and t % 2 == 0:
                    emit_l2_pair(pend.pop(0), pend.pop(0))
        for p_ in pend:
            emit_l2(p_)

    nc.finalize()
    return nc


def kernel(x, W1, b1, W2, b2):
    global LAST_RESULTS
    x = np.asarray(x, dtype=np.float32)
    W1 = np.asarray(W1, dtype=np.float32)
    b1 = np.asarray(b1, dtype=np.float32)
    W2 = np.asarray(W2, dtype=np.float32)
    b2 = np.asarray(b2, dtype=np.float32)

    xb = x.astype(BF16)
    # dense im2col phase layout (see module docstring): per (partition,
    # K-chunk) all 3969 window columns flat, window-row major; groups are
    # column ranges of this buffer
    xd = np.empty((B, 128, 2, NWIN), dtype=BF16)
    for p in range(128):
        kh, kwp = p // S, p % S
        for c in range(2):
            xs = xb[:, kh::S, kwp + S * c :: S][:, :OH, :OW]  # [B, 63, 63]
            xd[:, p, c, :] = xs.reshape(B, NWIN)

    # W1 row r = kh*16 + kw; chunk c, partition p=(kh*8+kwp) <- row kh*16 + 8*c + kwp
    w1p = (
        W1.reshape(KK, 2, S, HID).transpose(0, 2, 1, 3).reshape(128, 2, HID)
    ).astype(BF16)
    w2p = W2.reshape(NHB, 128).T.copy().astype(BF16)  # [p, hb] = W2[hb*128+p]
    b1p = b1.reshape(1, HID).astype(BF16)
    b1_nonzero = bool(np.any(b1 != 0.0))
    b2_val = float(b2.reshape(-1)[0])

    nc = _build_nc(b2_val, b1_nonzero)

    in_maps = []
    for c in range(NCORES):
        in_maps.append(
            {
                "x": np.ascontiguousarray(xd[c * BPC : (c + 1) * BPC]),
                "w1": w1p,
                "w2": w2p,
                "b1": b1p,
            }
        )

    LAST_RESULTS = run_bass_kernel_spmd(
        nc,
        in_maps,
        core_ids=list(range(NCORES)),
        trace=bool(int(os.environ.get("KERNEL_TRACE", "0") or "0")),
    )
    y = np.concatenate([r["y"] for r in LAST_RESULTS.results], axis=0)
    return y.astype(np.float32)

